# revision 27
# baseline (speedup 1.0000x reference)
"""MultiHeadAttention TRN2 Bass kernel.

Problem: B=4, S=2048, D=768, H=12 heads (DK=64).
Sharding: 8 cores = (batch b in 0..3) x (head-half in 0..1); each core
computes 6 heads of one batch element end-to-end (tensor-parallel over
heads within a batch). Host pre-transposes activations to [D, S] (and
casts to bf16 in the default fast path), slices projection weights per
head-half, and sums the two partial outputs per batch (+ bv@Wo + bo
correction, exact because softmax rows sum to 1).

Key optimization vs the dense formulation: the mask is per-key (same
for every query/head in a batch), so masked keys are removed ENTIRELY
on the host -- k/v are gathered down to the ~50% kept keys and padded
with zeros to SKV (multiple of 128, >= 1024). mv[s]=1 marks real keys,
0 marks padding; it is folded into vh_aug so padded keys contribute
exactly 0 to both the softmax numerator and denominator. This cuts
k/v-proj, scores, exp, and attn@V work by ~44% with bit-identical
semantics to the -inf mask.

On-core math (SKV = padded kept-key count, NKT = SKV/128):
  qh^T[384, S]: lhsT=Wq tile [Din,dout], rhs=q^T tile [Din,s] (+bq)
  kh^T[384, SKV] likewise; vh natural [SKV, 390] via lhsT=v^T, rhs=Wv:
    vh_aug[s, 65j..65j+64] = [mv(s)*vh_head_j(s, :), mv(s)]
  S^T[k, q] = kh_head^T.T @ qh_head^T  (contraction d=64; two heads of
    a pair go to disjoint PSUM halves of one [128,1024] tile)
  P^T = exp(S^T * 0.125)               (ACT, fused scale, no max-sub)
  ctx_aug^T[0:65, q] += vh_aug_j[kc].T @ P^T[kc]  over NKT k-chunks
    rows 0..63 = unnormalized ctx^T, row 64 = softmax denominator
  rs = reciprocal_approx_fast(denom); bcast on gpsimd;
  cn = ctx^T * rs   (drains deferred so the PE pipeline never waits)
  out[q, 768] = sum_dt cn[dt].T @ Wo tiles  (per 128-q chunk)
"""

import os
import sys
import types
from contextlib import ExitStack

import ml_dtypes
import numpy as np

import concourse.bacc as bacc
import concourse.bass as bass
import concourse.mybir as mybir
import concourse.tile as tile
from concourse import bass_utils
from concourse.bass import ts, ds

F32 = mybir.dt.float32
F32R = mybir.dt.float32r
BF16 = mybir.dt.bfloat16

D = 768        # model dim
DH = 384       # per-core head dim (6 heads x 64)
HPC = 6        # heads per core
VW = HPC * 65  # vh_aug free width (390)


def build_nc(S=2048, SKV=1152, bf16=True):
    nc = bacc.Bacc("TRN2", target_bir_lowering=False, debug=False)

    MMD = BF16 if bf16 else F32R    # matmul operand dtype
    NKT = SKV // 128                # 128-wide k-tiles
    assert SKV % 128 == 0 and NKT >= 8
    QBW = min(512, S)               # attention q-block width
    NQB = S // QBW                  # q blocks
    CWQ = min(1024, S)              # q-proj s-chunk width
    # k-proj free-dim chunk: largest 128*d <= 512 with d | NKT
    CWK = next(128 * d for d in (4, 3, 2, 1) if NKT % d == 0)

    qT = nc.dram_tensor("qT", [D, S], MMD, kind="ExternalInput").ap()
    kT = nc.dram_tensor("kT", [D, SKV], MMD, kind="ExternalInput").ap()
    vT = nc.dram_tensor("vT", [D, SKV], MMD, kind="ExternalInput").ap()
    wq = nc.dram_tensor("wq", [D, DH], MMD, kind="ExternalInput").ap()
    wk = nc.dram_tensor("wk", [D, DH], MMD, kind="ExternalInput").ap()
    wv = nc.dram_tensor("wv", [D, DH], MMD, kind="ExternalInput").ap()
    wo = nc.dram_tensor("wo", [DH, D], MMD, kind="ExternalInput").ap()
    # col 0..2 = bq (3 dt-tiles), 3..5 = bk, 6..6+NKT = mv (padding flag)
    smalls = nc.dram_tensor("smalls", [128, 6 + NKT], F32, kind="ExternalInput").ap()
    out = nc.dram_tensor("out", [S, D], BF16, kind="ExternalOutput").ap()

    with tile.TileContext(nc) as tc, ExitStack() as ctx:
        P = 128
        wpool = ctx.enter_context(tc.tile_pool(name="w", bufs=1))
        xin = ctx.enter_context(tc.tile_pool(name="xin", bufs=12))
        persist = ctx.enter_context(tc.tile_pool(name="persist", bufs=1))
        ppool = ctx.enter_context(tc.tile_pool(name="p", bufs=3))
        small = ctx.enter_context(tc.tile_pool(name="small", bufs=2))
        outp = ctx.enter_context(tc.tile_pool(name="outp", bufs=2))
        psA = ctx.enter_context(tc.tile_pool(name="psA", bufs=2, space="PSUM"))
        psB = ctx.enter_context(tc.tile_pool(name="psB", bufs=4, space="PSUM"))

        # Round-robin DMA issue across 4 engine sequencers: each dma_start
        # costs ~600ns of issue time on its engine, so spreading the ~45
        # phase-1 loads over 4 queues (in dependency order: wk+kt first)
        # cuts the serial descriptor-issue head from ~20us to ~4us.
        dmaq = [nc.sync, nc.gpsimd, nc.scalar]
        dqi = [0]

        def dq_start(dst, src):
            dmaq[dqi[0] % 3].dma_start(dst, src)
            dqi[0] += 1

        # ---- constants / small tensors ----
        wq_sb = [wpool.tile([P, DH], MMD, name=f"wq{c}", tag=f"wq{c}") for c in range(6)]
        wk_sb = [wpool.tile([P, DH], MMD, name=f"wk{c}", tag=f"wk{c}") for c in range(6)]
        wv_sb = [wpool.tile([P, DH], MMD, name=f"wv{c}", tag=f"wv{c}") for c in range(6)]
        wo_sb = [wpool.tile([P, D], MMD, name=f"wo{c}", tag=f"wo{c}") for c in range(3)]
        sm_sb = wpool.tile([128, 6 + NKT], F32, tag="smalls")
        # DMA priority order = compute order: q-proj runs first (so its data
        # loads first), k-proj next (kt fully landed by then -> no mid-kproj
        # DMA stalls that would reset the PE p-state), v/o/deferred-q last.
        # Each dma_start rides a single ~20GB/s hardware ring, so big loads
        # are split into ~128KB pieces to spread across the 16 rings.
        qt0 = [xin.tile([P, CWQ], MMD, name="xin", tag="xin") for c in range(6)]
        for c in range(6):
            dq_start(wq_sb[c][:], wq[ts(c, P), :])
            dq_start(qt0[c][:, 0:512], qT[ts(c, P), 0:512])
        kt = [xin.tile([P, SKV], MMD, name="xin", tag="xin") for c in range(6)]
        HK = SKV // 2
        for c in range(6):
            dq_start(wk_sb[c][:], wk[ts(c, P), :])
            dq_start(kt[c][:, 0:HK], kT[ts(c, P), 0:HK])
            dq_start(kt[c][:, HK:SKV], kT[ts(c, P), HK:SKV])
        for c in range(6):
            dq_start(qt0[c][:, 512:CWQ], qT[ts(c, P), 512:CWQ])
        dq_start(sm_sb[:], smalls[:, :])
        bq_sb = [sm_sb[:, t : t + 1] for t in range(3)]
        bk_sb = [sm_sb[:, 3 + t : 4 + t] for t in range(3)]
        mv_sb = [sm_sb[:, 6 + st : 7 + st] for st in range(NKT)]
        ones6 = wpool.tile([P, HPC], F32, tag="ones6")
        nc.vector.memset(ones6[:], 1.0)
        vt = [xin.tile([P, SKV], MMD, name="xin", tag="xin") for c in range(6)]
        for c in range(6):
            dq_start(wv_sb[c][:], wv[ts(c, P), :])
            dq_start(vt[c][:, 0:HK], vT[ts(c, P), 0:HK])
            dq_start(vt[c][:, HK:SKV], vT[ts(c, P), HK:SKV])
        for c in range(3):
            dq_start(wo_sb[c][:], wo[ts(c, P), :])
        qproj_xt = {0: qt0}
        for sc in range(1, S // CWQ):
            qproj_xt[sc] = [
                xin.tile([P, CWQ], MMD, name="xin", tag="xin") for c in range(6)
            ]
            for c in range(6):
                dq_start(qproj_xt[sc][c][:, 0:512], qT[ts(c, P), ds(sc * CWQ, 512)])
                dq_start(
                    qproj_xt[sc][c][:, 512:CWQ],
                    qT[ts(c, P), ds(sc * CWQ + 512, 512)],
                )

        # PE warm-up: the tensor engine ramps 0.65->1.2->2.4GHz over ~3us of
        # CONTINUOUS work; any idle gap resets it. Dummy matmuls on a zeroed
        # tile bridge the initial DMA wait so q-proj starts at full clock.
        warm = wpool.tile([P, 512], MMD, tag="warm")
        nc.vector.memset(warm[:], 0.0)
        wps = psA.tile([P, 512], F32, name="psA", tag="psA")
        for _ in range(14):
            nc.tensor.matmul(
                wps[:], lhsT=warm[:, 0:128], rhs=warm[:], start=True, stop=True
            )

        # ---- persistent activations ----
        khT = [persist.tile([P, SKV], MMD, name=f"khT{t}", tag=f"khT{t}") for t in range(3)]
        qhT = [persist.tile([P, S], MMD, name=f"qhT{t}", tag=f"qhT{t}") for t in range(3)]
        vh = [persist.tile([P, VW], MMD, name=f"vh{st}", tag=f"vh{st}") for st in range(NKT)]
        cn = [persist.tile([P, S], MMD, name=f"cn{t}", tag=f"cn{t}") for t in range(3)]

        # ---- phase 1 (minimal): q-proj of first 512 cols, k-proj, v-proj of
        # the first 3 k-tiles. Everything else runs as phase-2 filler.
        for dt in range(3):
            ps = psA.tile([P, 512], F32, name="psA", tag="psA")
            for c in range(6):
                nc.tensor.matmul(
                    ps[:],
                    lhsT=wq_sb[c][:, ts(dt, P)],
                    rhs=qt0[c][:, ts(0, 512)],
                    start=(c == 0),
                    stop=(c == 5),
                )
            nc.vector.tensor_scalar_add(
                out=qhT[dt][:, ts(0, 512)], in0=ps[:],
                scalar1=bq_sb[dt],
            )

        for sc in range(SKV // CWK):
            for dt in range(3):
                ps = psA.tile([P, CWK], F32, name="psA", tag="psA")
                for c in range(6):
                    nc.tensor.matmul(
                        ps[:],
                        lhsT=wk_sb[c][:, ts(dt, P)],
                        rhs=kt[c][:, ts(sc, CWK)],
                        start=(c == 0),
                        stop=(c == 5),
                    )
                nc.vector.tensor_scalar_add(
                    out=khT[dt][:, ts(sc, CWK)], in0=ps[:],
                    scalar1=bk_sb[dt],
                )

        def vproj_sub(st):
            ps = psB.tile([P, 512], F32, name="psB", tag="psB")
            for c in range(6):
                nc.tensor.matmul(
                    ps[:, :DH],
                    lhsT=vt[c][:, ts(st, P)],
                    rhs=wv_sb[c][:],
                    start=(c == 0),
                    stop=(c == 5),
                )
            vh3 = vh[st].rearrange("p (h c) -> p h c", c=65)
            nc.vector.tensor_scalar_mul(
                out=vh3[:, :, 0:64],
                in0=ps[:, :DH].rearrange("p (h c) -> p h c", c=64),
                scalar1=mv_sb[st],
            )
            nc.vector.tensor_scalar_mul(
                out=vh3[:, :, 64:65],
                in0=ones6[:].rearrange("p (h c) -> p h c", c=1),
                scalar1=mv_sb[st],
            )

        for st in range(NKT):
            vproj_sub(st)
        pend_vproj = []
        pend_qproj = [(0, dt, 1) for dt in range(3)] + [
            (sc, dt, u)
            for sc in range(1, S // CWQ)
            for dt in range(3)
            for u in range(CWQ // 512)
        ]

        # ---- phase 2: attention, head-pair steps ----
        # Each step handles BOTH heads of a pair for one k-chunk: the two
        # scores matmuls live in disjoint PE row groups (base partition 0
        # and 64) and share one [128,1024] PSUM tile (head A in cols 0:512,
        # head B in 512:1024) -> one exp per step. Scores run 2 steps ahead
        # of attn@V. Drains and filler work (deferred q-proj, O-proj) go to
        # psB spare slots, never stalling the scores pipeline on psA.
        hq = [(pr, qb) for qb in range(NQB) for pr in range(3)]
        steps = [(pr, qb, kc) for (pr, qb) in hq for kc in range(NKT)]

        ctx_ps = {}
        st_ps = {}

        def scores(pr, qb, kc):
            ps = psA.tile([P, 1024], F32, name="psA", tag="psA")
            for hh in range(2):
                nc.tensor.matmul(
                    ps[:, ts(hh, 512)],
                    lhsT=khT[pr][64 * hh : 64 * hh + 64, ts(kc, P)],
                    rhs=qhT[pr][64 * hh : 64 * hh + 64, ts(qb, QBW)],
                    start=True,
                    stop=True,
                )
            st_ps[(pr, qb, kc)] = ps

        def attnv(pr, qb, kc, pt):
            for hh in range(2):
                h = 2 * pr + hh
                nc.tensor.matmul(
                    ctx_ps[(h, qb)][0:65, :],
                    lhsT=vh[kc][:, ds(65 * h, 65)],
                    rhs=pt[:, ts(hh, 512)],
                    start=(kc == 0),
                    stop=(kc == NKT - 1),
                )

        def drain(h, qb):
            """Normalize + store ctx for a finished (h, qb)."""
            dt, pb = h // 2, 64 * (h % 2)
            cps = ctx_ps.pop((h, qb))
            den = small.tile([1, QBW], F32, name="den", tag="den")
            nc.vector.tensor_copy(den[:], cps[64:65, :])
            rs = small.tile([1, QBW], F32, name="rs", tag="rs")
            nc.vector.reciprocal_approx_fast(rs[:], den[:])
            bcs = small.tile([64, QBW], F32, name="bcs", tag="bcs")
            nc.gpsimd.partition_broadcast(bcs[:], rs[:])
            if pb == 0:
                nc.vector.tensor_tensor(
                    out=cn[dt][0:64, ts(qb, QBW)],
                    in0=cps[0:64, :],
                    in1=bcs[:],
                    op=mybir.AluOpType.mult,
                )
            else:
                tmp = small.tile([64, QBW], MMD, name="tmp", tag="tmp")
                nc.vector.tensor_tensor(
                    out=tmp[:], in0=cps[0:64, :], in1=bcs[:],
                    op=mybir.AluOpType.mult,
                )
                nc.gpsimd.dma_start(cn[dt][64:128, ts(qb, QBW)], tmp[:])

        oq = [nc.sync, nc.gpsimd]

        def oproj(qc):
            ups = psB.tile([P, 512], F32, name="psB", tag="psB")
            ups2 = psB.tile([P, 256], F32, name="psB2", tag="psB")
            for ps_, n0, nw in ((ups, 0, 512), (ups2, 512, 256)):
                for dt in range(3):
                    nc.tensor.matmul(
                        ps_[:, 0:nw],
                        lhsT=cn[dt][:, ts(qc, P)],
                        rhs=wo_sb[dt][:, ds(n0, nw)],
                        start=(dt == 0),
                        stop=(dt == 2),
                    )
            ot = outp.tile([P, D], MMD, name="ot", tag="ot")
            nc.vector.tensor_copy(ot[:, 0:512], ups[:, 0:512])
            oq[qc % 2].dma_start(out[ts(qc, P), 0:512], ot[:, 0:512])
            nc.vector.tensor_copy(ot[:, 512:768], ups2[:, 0:256])
            oq[(qc + 1) % 2].dma_start(out[ts(qc, P), 512:768], ot[:, 512:768])

        def qproj_sub(sc, dt, u):
            ps = psB.tile([P, 512], F32, name="psB", tag="psB")
            for c in range(6):
                nc.tensor.matmul(
                    ps[:],
                    lhsT=wq_sb[c][:, ts(dt, P)],
                    rhs=qproj_xt[sc][c][:, ts(u, 512)],
                    start=(c == 0),
                    stop=(c == 5),
                )
            nc.vector.tensor_scalar_add(
                out=qhT[dt][:, ds(sc * CWQ + u * 512, 512)], in0=ps[:],
                scalar1=bq_sb[dt],
            )
            if dt == 2 and u == CWQ // 512 - 1:
                qproj_xt.pop(sc)

        DEPTH = 2
        pend_drain = []
        pend_oproj = []
        for n, (pr, qb, kc) in enumerate(steps):
            if kc == 0:
                for hh in range(2):
                    ctx_ps[(2 * pr + hh, qb)] = psB.tile(
                        [P, QBW], F32, name="psB", tag="psB"
                    )[0:65, :]
            if n < DEPTH:
                scores(*steps[n])
            pt = ppool.tile([P, 1024], MMD, name="pt", tag="pt")
            nc.scalar.activation(
                pt[:], st_ps.pop((pr, qb, kc))[:],
                mybir.ActivationFunctionType.Exp, scale=0.125,
            )
            if n + DEPTH < len(steps):
                scores(*steps[n + DEPTH])
            attnv(pr, qb, kc, pt)
            if pend_vproj and kc < 6:
                vproj_sub(pend_vproj.pop(0))
            elif kc in (2, 4) and pend_drain:
                hd, qd = pend_drain.pop(0)
                drain(hd, qd)
                if hd == HPC - 1:
                    pend_oproj.extend(range(qd * (QBW // P), (qd + 1) * (QBW // P)))
            elif pend_qproj and kc in (6, NKT - 1):
                qproj_sub(*pend_qproj.pop(0))
            elif pend_oproj and kc in (5, 6, 7, NKT - 1):
                oproj(pend_oproj.pop(0))
            if kc == NKT - 1:
                pend_drain.extend([(2 * pr, qb), (2 * pr + 1, qb)])
        # tail: batch the final drains phase-by-phase so the two DVE chains
        # and the two gpsimd broadcasts interleave instead of serializing
        infos = []
        for hd, qd in pend_drain:
            dt, pb = hd // 2, 64 * (hd % 2)
            cps = ctx_ps.pop((hd, qd))
            den = small.tile([1, QBW], F32, name="den", tag="den")
            nc.vector.tensor_copy(den[:], cps[64:65, :])
            rs = small.tile([1, QBW], F32, name="rs", tag="rs")
            nc.vector.reciprocal_approx_fast(rs[:], den[:])
            bcs = small.tile([64, QBW], F32, name="bcs", tag="bcs")
            nc.gpsimd.partition_broadcast(bcs[:], rs[:])
            infos.append((hd, qd, dt, pb, cps, bcs))
        for hd, qd, dt, pb, cps, bcs in infos:
            if pb == 0:
                nc.vector.tensor_tensor(
                    out=cn[dt][0:64, ts(qd, QBW)],
                    in0=cps[0:64, :], in1=bcs[:],
                    op=mybir.AluOpType.mult,
                )
            else:
                tmp = small.tile([64, QBW], MMD, name="tmp", tag="tmp")
                nc.vector.tensor_tensor(
                    out=tmp[:], in0=cps[0:64, :], in1=bcs[:],
                    op=mybir.AluOpType.mult,
                )
                nc.gpsimd.dma_start(cn[dt][64:128, ts(qd, QBW)], tmp[:])
            if hd == HPC - 1:
                pend_oproj.extend(range(qd * (QBW // P), (qd + 1) * (QBW // P)))
        for qc in pend_oproj:
            oproj(qc)

    nc.compile()
    return nc


_NC_CACHE = {}


def _get_nc(S, SKV, bf16=True):
    key = (S, SKV, bf16)
    if key not in _NC_CACHE:
        _NC_CACHE[key] = build_nc(S, SKV, bf16)
    return _NC_CACHE[key]


def _install_ntff_hook():
    try:
        mod = types.ModuleType("antenv.axon_hooks")
        state = {"hook": None}
        mod.set_axon_ntff_profile_hook = lambda h: state.__setitem__("hook", h)
        mod.get_axon_ntff_profile_hook = lambda: state["hook"]
        sys.modules["antenv.axon_hooks"] = mod
        from trn_agent_boot.trn_boot import _ntff_profile_via_ctypes

        mod.set_axon_ntff_profile_hook(
            _ntff_profile_via_ctypes("/opt/axon/libaxon_pjrt.so")
        )
        bass_utils.upload_artifacts = lambda tmpdir: "local://" + tmpdir
        return state["hook"] is not None
    except Exception:
        return False


def run_cores(in_maps, S=2048, SKV=1152, bf16=True, profile=False):
    nc = _get_nc(S, SKV, bf16)
    trace = bool(profile) and _install_ntff_hook()
    res = bass_utils.run_bass_kernel_spmd(
        nc, in_maps, core_ids=list(range(len(in_maps))), trace=trace
    )
    return res


def make_in_maps(q, k, v, mask, Wq, bq, Wk, bk, Wv, Wo, bf16=True):
    B, S, _ = q.shape
    mmd = ml_dtypes.bfloat16 if bf16 else np.float32
    q = np.asarray(q, np.float32)
    k = np.asarray(k, np.float32)
    v = np.asarray(v, np.float32)
    keep = ~np.asarray(mask).reshape(B, S)
    counts = keep.sum(axis=1)
    SKV = max(1024, int(-(-int(counts.max()) // 128)) * 128)
    Wq, Wk, Wv, Wo = (np.asarray(a, np.float32) for a in (Wq, Wk, Wv, Wo))
    bq, bk = np.asarray(bq, np.float32), np.asarray(bk, np.float32)
    in_maps = []
    NKT = SKV // 128
    for b in range(B):
        idx = np.nonzero(keep[b])[0]
        n = len(idx)
        kTc = np.zeros((D, SKV), np.float32)
        kTc[:, :n] = k[b][idx].T
        vTc = np.zeros((D, SKV), np.float32)
        vTc[:, :n] = v[b][idx].T
        mvec = np.zeros(SKV, np.float32)
        mvec[:n] = 1.0
        qTb = np.ascontiguousarray(q[b].T).astype(mmd)
        kTc = kTc.astype(mmd)
        vTc = vTc.astype(mmd)
        for half in range(2):
            hs = slice(DH * half, DH * (half + 1))
            # smalls: col 0..2 bq dt-tiles, 3..5 bk, 6.. mv k-tiles
            sm = np.zeros((128, 6 + NKT), np.float32)
            sm[:, 0:3] = bq[hs].reshape(3, 128).T
            sm[:, 3:6] = bk[hs].reshape(3, 128).T
            sm[:, 6:] = mvec.reshape(NKT, 128).T
            in_maps.append(
                {
                    "qT": qTb,
                    "kT": kTc,
                    "vT": vTc,
                    "wq": np.ascontiguousarray(Wq[:, hs]).astype(mmd),
                    "wk": np.ascontiguousarray(Wk[:, hs]).astype(mmd),
                    "wv": np.ascontiguousarray(Wv[:, hs]).astype(mmd),
                    "wo": np.ascontiguousarray(Wo[hs, :]).astype(mmd),
                    "smalls": sm,
                }
            )
    return in_maps, SKV


def kernel(q, k, v, mask, Wq, bq, Wk, bk, Wv, bv, Wo, bo):
    q = np.asarray(q, np.float32)
    B, S, _ = q.shape
    bf16 = os.environ.get("BASS_PRECISE") != "1"
    in_maps, SKV = make_in_maps(q, k, v, mask, Wq, bq, Wk, bk, Wv, Wo, bf16=bf16)
    res = run_cores(
        in_maps, S=S, SKV=SKV, bf16=bf16,
        profile=os.environ.get("BASS_PROFILE") == "1",
    )
    if os.environ.get("BASS_PROFILE") == "1" and res.exec_time_ns is not None:
        print(f"HW exec time: {res.exec_time_ns} ns")
    cvec = (
        np.asarray(bv, np.float32) @ np.asarray(Wo, np.float32)
        + np.asarray(bo, np.float32)
    )
    out = np.empty((B, S, D), np.float32)
    for b in range(B):
        out[b] = (
            np.asarray(res.results[2 * b]["out"], np.float32)
            + np.asarray(res.results[2 * b + 1]["out"], np.float32)
            + cvec
        )
    return out


# revision 30
# speedup vs baseline: 1.2533x; 1.2533x over previous
"""MultiHeadAttention TRN2 Bass kernel.

Problem: B=4, S=2048, D=768, H=12 heads (DK=64).
Sharding: 8 cores = (batch b in 0..3) x (head-half in 0..1); each core
computes 6 heads of one batch element end-to-end (tensor-parallel over
heads within a batch). Host pre-transposes activations to [D, S] (and
casts to bf16 in the default fast path), slices projection weights per
head-half, and sums the two partial outputs per batch (+ bv@Wo + bo
correction, exact because softmax rows sum to 1).

Key optimization vs the dense formulation: the mask is per-key (same
for every query/head in a batch), so masked keys are removed ENTIRELY
on the host -- k/v are gathered down to the ~50% kept keys and padded
with zeros to SKV (multiple of 128, >= 1024). mv[s]=1 marks real keys,
0 marks padding; it is folded into vh_aug so padded keys contribute
exactly 0 to both the softmax numerator and denominator. This cuts
k/v-proj, scores, exp, and attn@V work by ~44% with bit-identical
semantics to the -inf mask.

On-core math (SKV = padded kept-key count, NKT = SKV/128):
  qh^T[384, S]: lhsT=Wq tile [Din,dout], rhs=q^T tile [Din,s] (+bq)
  kh^T[384, SKV] likewise; vh natural [SKV, 390] via lhsT=v^T, rhs=Wv:
    vh_aug[s, 65j..65j+64] = [mv(s)*vh_head_j(s, :), mv(s)]
  S^T[k, q] = kh_head^T.T @ qh_head^T  (contraction d=64; two heads of
    a pair go to disjoint PSUM halves of one [128,1024] tile)
  P^T = exp(S^T * 0.125)               (ACT, fused scale, no max-sub)
  ctx_aug^T[0:65, q] += vh_aug_j[kc].T @ P^T[kc]  over NKT k-chunks
    rows 0..63 = unnormalized ctx^T, row 64 = softmax denominator
  rs = reciprocal_approx_fast(denom); bcast on gpsimd;
  cn = ctx^T * rs   (drains deferred so the PE pipeline never waits)
  out[q, 768] = sum_dt cn[dt].T @ Wo tiles  (per 128-q chunk)
"""

import os
import sys
import types
from contextlib import ExitStack

import ml_dtypes
import numpy as np

import concourse.bacc as bacc
import concourse.bass as bass
import concourse.mybir as mybir
import concourse.tile as tile
from concourse import bass_utils
from concourse.bass import ts, ds

F32 = mybir.dt.float32
F32R = mybir.dt.float32r
BF16 = mybir.dt.bfloat16

D = 768        # model dim
DH = 384       # per-core head dim (6 heads x 64)
HPC = 6        # heads per core
VW = HPC * 65  # vh_aug free width (390)


def build_nc(S=2048, SKV=1152, bf16=True):
    nc = bacc.Bacc("TRN2", target_bir_lowering=False, debug=False)

    MMD = BF16 if bf16 else F32R    # matmul operand dtype
    NKT = SKV // 128                # 128-wide k-tiles
    assert SKV % 128 == 0 and NKT >= 8
    QBW = min(512, S)               # attention q-block width
    NQB = S // QBW                  # q blocks
    CWQ = min(1024, S)              # q-proj s-chunk width
    # k-proj free-dim chunk: largest 128*d <= 512 with d | NKT
    CWK = next(128 * d for d in (4, 3, 2, 1) if NKT % d == 0)

    qT = nc.dram_tensor("qT", [D, S], MMD, kind="ExternalInput").ap()
    kT = nc.dram_tensor("kT", [D, SKV], MMD, kind="ExternalInput").ap()
    vT = nc.dram_tensor("vT", [D, SKV], MMD, kind="ExternalInput").ap()
    wq = nc.dram_tensor("wq", [D, DH], MMD, kind="ExternalInput").ap()
    wk = nc.dram_tensor("wk", [D, DH], MMD, kind="ExternalInput").ap()
    wv = nc.dram_tensor("wv", [D, DH], MMD, kind="ExternalInput").ap()
    wo = nc.dram_tensor("wo", [DH, D], MMD, kind="ExternalInput").ap()
    # col 0..2 = bq (3 dt-tiles), 3..5 = bk, 6..6+NKT = mv (padding flag)
    smalls = nc.dram_tensor("smalls", [128, 6 + NKT], F32, kind="ExternalInput").ap()
    out = nc.dram_tensor("out", [S, D], BF16, kind="ExternalOutput").ap()

    with tile.TileContext(nc) as tc, ExitStack() as ctx:
        P = 128
        wpool = ctx.enter_context(tc.tile_pool(name="w", bufs=1))
        xin = ctx.enter_context(tc.tile_pool(name="xin", bufs=12))
        persist = ctx.enter_context(tc.tile_pool(name="persist", bufs=1))
        ppool = ctx.enter_context(tc.tile_pool(name="p", bufs=3))
        small = ctx.enter_context(tc.tile_pool(name="small", bufs=2))
        outp = ctx.enter_context(tc.tile_pool(name="outp", bufs=2))
        psA = ctx.enter_context(tc.tile_pool(name="psA", bufs=2, space="PSUM"))
        psB = ctx.enter_context(tc.tile_pool(name="psB", bufs=4, space="PSUM"))

        # Round-robin DMA issue across 4 engine sequencers: each dma_start
        # costs ~600ns of issue time on its engine, so spreading the ~45
        # phase-1 loads over 4 queues (in dependency order: wk+kt first)
        # cuts the serial descriptor-issue head from ~20us to ~4us.
        dmaq = [nc.sync, nc.gpsimd, nc.scalar]
        dqi = [0]

        def dq_start(dst, src):
            dmaq[dqi[0] % 3].dma_start(dst, src)
            dqi[0] += 1

        # ---- constants / small tensors ----
        wq_sb = [wpool.tile([P, DH], MMD, name=f"wq{c}", tag=f"wq{c}") for c in range(6)]
        wk_sb = [wpool.tile([P, DH], MMD, name=f"wk{c}", tag=f"wk{c}") for c in range(6)]
        wv_sb = [wpool.tile([P, DH], MMD, name=f"wv{c}", tag=f"wv{c}") for c in range(6)]
        wo_sb = [wpool.tile([P, D], MMD, name=f"wo{c}", tag=f"wo{c}") for c in range(3)]
        sm_sb = wpool.tile([128, 6 + NKT], F32, tag="smalls")
        # DMA priority order = compute order: q-proj runs first (so its data
        # loads first), k-proj next (kt fully landed by then -> no mid-kproj
        # DMA stalls that would reset the PE p-state), v/o/deferred-q last.
        # Each dma_start rides a single ~20GB/s hardware ring, so big loads
        # are split into ~128-150KB pieces to spread across the 16 rings,
        # issued in the order compute consumes them (kproj, vproj, qproj).
        kt = [xin.tile([P, SKV], MMD, name="xin", tag="xin") for c in range(6)]
        HK = SKV // 2
        for c in range(6):
            dq_start(wk_sb[c][:], wk[ts(c, P), :])
            dq_start(kt[c][:, 0:HK], kT[ts(c, P), 0:HK])
            dq_start(kt[c][:, HK:SKV], kT[ts(c, P), HK:SKV])
        dq_start(sm_sb[:], smalls[:, :])
        bq_sb = [sm_sb[:, t : t + 1] for t in range(3)]
        bk_sb = [sm_sb[:, 3 + t : 4 + t] for t in range(3)]
        mv_sb = [sm_sb[:, 6 + st : 7 + st] for st in range(NKT)]
        ones6 = wpool.tile([P, HPC], F32, tag="ones6")
        nc.vector.memset(ones6[:], 1.0)
        vt = [xin.tile([P, SKV], MMD, name="xin", tag="xin") for c in range(6)]
        for c in range(6):
            dq_start(wv_sb[c][:], wv[ts(c, P), :])
            dq_start(vt[c][:, 0:HK], vT[ts(c, P), 0:HK])
            dq_start(vt[c][:, HK:SKV], vT[ts(c, P), HK:SKV])
        qt0 = [xin.tile([P, CWQ], MMD, name="xin", tag="xin") for c in range(6)]
        for c in range(6):
            dq_start(wq_sb[c][:], wq[ts(c, P), :])
            dq_start(qt0[c][:, 0:512], qT[ts(c, P), 0:512])
        for c in range(6):
            dq_start(qt0[c][:, 512:CWQ], qT[ts(c, P), 512:CWQ])
        for c in range(3):
            dq_start(wo_sb[c][:], wo[ts(c, P), :])
        qproj_xt = {0: qt0}
        for sc in range(1, S // CWQ):
            qproj_xt[sc] = [
                xin.tile([P, CWQ], MMD, name="xin", tag="xin") for c in range(6)
            ]
            for c in range(6):
                dq_start(qproj_xt[sc][c][:, 0:512], qT[ts(c, P), ds(sc * CWQ, 512)])
                dq_start(
                    qproj_xt[sc][c][:, 512:CWQ],
                    qT[ts(c, P), ds(sc * CWQ + 512, 512)],
                )

        # ---- persistent activations ----
        khT = [persist.tile([P, SKV], MMD, name=f"khT{t}", tag=f"khT{t}") for t in range(3)]
        qhT = [persist.tile([P, S], MMD, name=f"qhT{t}", tag=f"qhT{t}") for t in range(3)]
        vh = [persist.tile([P, VW], MMD, name=f"vh{st}", tag=f"vh{st}") for st in range(NKT)]
        cn = [persist.tile([P, S], MMD, name=f"cn{t}", tag=f"cn{t}") for t in range(3)]

        # ---- phase 1: k-proj, v-proj, then q-proj of the first 512 cols
        # (matching DMA arrival order); the rest of q-proj is phase-2 filler.
        for sc in range(SKV // CWK):
            for dt in range(3):
                ps = psA.tile([P, CWK], F32, name="psA", tag="psA")
                for c in range(6):
                    nc.tensor.matmul(
                        ps[:],
                        lhsT=wk_sb[c][:, ts(dt, P)],
                        rhs=kt[c][:, ts(sc, CWK)],
                        start=(c == 0),
                        stop=(c == 5),
                    )
                nc.vector.tensor_scalar_add(
                    out=khT[dt][:, ts(sc, CWK)], in0=ps[:],
                    scalar1=bk_sb[dt],
                )

        def vproj_sub(st):
            ps = psB.tile([P, 512], F32, name="psB", tag="psB")
            for c in range(6):
                nc.tensor.matmul(
                    ps[:, :DH],
                    lhsT=vt[c][:, ts(st, P)],
                    rhs=wv_sb[c][:],
                    start=(c == 0),
                    stop=(c == 5),
                )
            vh3 = vh[st].rearrange("p (h c) -> p h c", c=65)
            nc.vector.tensor_scalar_mul(
                out=vh3[:, :, 0:64],
                in0=ps[:, :DH].rearrange("p (h c) -> p h c", c=64),
                scalar1=mv_sb[st],
            )
            nc.vector.tensor_scalar_mul(
                out=vh3[:, :, 64:65],
                in0=ones6[:].rearrange("p (h c) -> p h c", c=1),
                scalar1=mv_sb[st],
            )

        for st in range(NKT):
            vproj_sub(st)
        pend_vproj = []

        for dt in range(3):
            ps = psA.tile([P, 512], F32, name="psA", tag="psA")
            for c in range(6):
                nc.tensor.matmul(
                    ps[:],
                    lhsT=wq_sb[c][:, ts(dt, P)],
                    rhs=qt0[c][:, ts(0, 512)],
                    start=(c == 0),
                    stop=(c == 5),
                )
            nc.vector.tensor_scalar_add(
                out=qhT[dt][:, ts(0, 512)], in0=ps[:],
                scalar1=bq_sb[dt],
            )
        pend_qproj = [(0, dt, 1) for dt in range(3)] + [
            (sc, dt, u)
            for sc in range(1, S // CWQ)
            for dt in range(3)
            for u in range(CWQ // 512)
        ]

        # ---- phase 2: attention, head-pair steps ----
        # Each step handles BOTH heads of a pair for one k-chunk: the two
        # scores matmuls live in disjoint PE row groups (base partition 0
        # and 64) and share one [128,1024] PSUM tile (head A in cols 0:512,
        # head B in 512:1024) -> one exp per step. Scores run 2 steps ahead
        # of attn@V. Drains and filler work (deferred q-proj, O-proj) go to
        # psB spare slots, never stalling the scores pipeline on psA.
        hq = [(pr, qb) for qb in range(NQB) for pr in range(3)]
        steps = [(pr, qb, kc) for (pr, qb) in hq for kc in range(NKT)]

        ctx_ps = {}
        st_ps = {}

        def scores(pr, qb, kc):
            ps = psA.tile([P, 1024], F32, name="psA", tag="psA")
            for hh in range(2):
                nc.tensor.matmul(
                    ps[:, ts(hh, 512)],
                    lhsT=khT[pr][64 * hh : 64 * hh + 64, ts(kc, P)],
                    rhs=qhT[pr][64 * hh : 64 * hh + 64, ts(qb, QBW)],
                    start=True,
                    stop=True,
                )
            st_ps[(pr, qb, kc)] = ps

        def attnv(pr, qb, kc, pt):
            for hh in range(2):
                h = 2 * pr + hh
                nc.tensor.matmul(
                    ctx_ps[(h, qb)][0:65, :],
                    lhsT=vh[kc][:, ds(65 * h, 65)],
                    rhs=pt[:, ts(hh, 512)],
                    start=(kc == 0),
                    stop=(kc == NKT - 1),
                )

        def drain(h, qb):
            """Normalize + store ctx for a finished (h, qb)."""
            dt, pb = h // 2, 64 * (h % 2)
            cps = ctx_ps.pop((h, qb))
            den = small.tile([1, QBW], F32, name="den", tag="den")
            nc.vector.tensor_copy(den[:], cps[64:65, :])
            rs = small.tile([1, QBW], F32, name="rs", tag="rs")
            nc.vector.reciprocal_approx_fast(rs[:], den[:])
            bcs = small.tile([64, QBW], F32, name="bcs", tag="bcs")
            nc.gpsimd.partition_broadcast(bcs[:], rs[:])
            if pb == 0:
                nc.vector.tensor_tensor(
                    out=cn[dt][0:64, ts(qb, QBW)],
                    in0=cps[0:64, :],
                    in1=bcs[:],
                    op=mybir.AluOpType.mult,
                )
            else:
                tmp = small.tile([64, QBW], MMD, name="tmp", tag="tmp")
                nc.vector.tensor_tensor(
                    out=tmp[:], in0=cps[0:64, :], in1=bcs[:],
                    op=mybir.AluOpType.mult,
                )
                nc.gpsimd.dma_start(cn[dt][64:128, ts(qb, QBW)], tmp[:])

        oq = [nc.sync, nc.gpsimd]

        def oproj(qc):
            ups = psB.tile([P, 512], F32, name="psB", tag="psB")
            ups2 = psB.tile([P, 256], F32, name="psB2", tag="psB")
            for ps_, n0, nw in ((ups, 0, 512), (ups2, 512, 256)):
                for dt in range(3):
                    nc.tensor.matmul(
                        ps_[:, 0:nw],
                        lhsT=cn[dt][:, ts(qc, P)],
                        rhs=wo_sb[dt][:, ds(n0, nw)],
                        start=(dt == 0),
                        stop=(dt == 2),
                    )
            ot = outp.tile([P, D], MMD, name="ot", tag="ot")
            nc.vector.tensor_copy(ot[:, 0:512], ups[:, 0:512])
            oq[qc % 2].dma_start(out[ts(qc, P), 0:512], ot[:, 0:512])
            nc.vector.tensor_copy(ot[:, 512:768], ups2[:, 0:256])
            oq[(qc + 1) % 2].dma_start(out[ts(qc, P), 512:768], ot[:, 512:768])

        def qproj_sub(sc, dt, u):
            ps = psB.tile([P, 512], F32, name="psB", tag="psB")
            for c in range(6):
                nc.tensor.matmul(
                    ps[:],
                    lhsT=wq_sb[c][:, ts(dt, P)],
                    rhs=qproj_xt[sc][c][:, ts(u, 512)],
                    start=(c == 0),
                    stop=(c == 5),
                )
            nc.vector.tensor_scalar_add(
                out=qhT[dt][:, ds(sc * CWQ + u * 512, 512)], in0=ps[:],
                scalar1=bq_sb[dt],
            )
            if dt == 2 and u == CWQ // 512 - 1:
                qproj_xt.pop(sc)

        DEPTH = 2
        pend_drain = []
        pend_oproj = []
        for n, (pr, qb, kc) in enumerate(steps):
            if kc == 0:
                for hh in range(2):
                    ctx_ps[(2 * pr + hh, qb)] = psB.tile(
                        [P, QBW], F32, name="psB", tag="psB"
                    )[0:65, :]
            if n < DEPTH:
                scores(*steps[n])
            pt = ppool.tile([P, 1024], MMD, name="pt", tag="pt")
            nc.scalar.activation(
                pt[:], st_ps.pop((pr, qb, kc))[:],
                mybir.ActivationFunctionType.Exp, scale=0.125,
            )
            if n + DEPTH < len(steps):
                scores(*steps[n + DEPTH])
            attnv(pr, qb, kc, pt)
            if pend_vproj and kc < 6:
                vproj_sub(pend_vproj.pop(0))
            elif kc in (2, 4) and pend_drain:
                hd, qd = pend_drain.pop(0)
                drain(hd, qd)
                if hd == HPC - 1:
                    pend_oproj.extend(range(qd * (QBW // P), (qd + 1) * (QBW // P)))
            elif pend_qproj and kc in (6, NKT - 1):
                qproj_sub(*pend_qproj.pop(0))
            elif pend_oproj and kc in (5, 6, 7, NKT - 1):
                oproj(pend_oproj.pop(0))
            if kc == NKT - 1:
                pend_drain.extend([(2 * pr, qb), (2 * pr + 1, qb)])
        # tail: batch the final drains phase-by-phase so the two DVE chains
        # and the two gpsimd broadcasts interleave instead of serializing
        infos = []
        for hd, qd in pend_drain:
            dt, pb = hd // 2, 64 * (hd % 2)
            cps = ctx_ps.pop((hd, qd))
            den = small.tile([1, QBW], F32, name="den", tag="den")
            nc.vector.tensor_copy(den[:], cps[64:65, :])
            rs = small.tile([1, QBW], F32, name="rs", tag="rs")
            nc.vector.reciprocal_approx_fast(rs[:], den[:])
            bcs = small.tile([64, QBW], F32, name="bcs", tag="bcs")
            nc.gpsimd.partition_broadcast(bcs[:], rs[:])
            infos.append((hd, qd, dt, pb, cps, bcs))
        for hd, qd, dt, pb, cps, bcs in infos:
            if pb == 0:
                nc.vector.tensor_tensor(
                    out=cn[dt][0:64, ts(qd, QBW)],
                    in0=cps[0:64, :], in1=bcs[:],
                    op=mybir.AluOpType.mult,
                )
            else:
                tmp = small.tile([64, QBW], MMD, name="tmp", tag="tmp")
                nc.vector.tensor_tensor(
                    out=tmp[:], in0=cps[0:64, :], in1=bcs[:],
                    op=mybir.AluOpType.mult,
                )
                nc.gpsimd.dma_start(cn[dt][64:128, ts(qd, QBW)], tmp[:])
            if hd == HPC - 1:
                pend_oproj.extend(range(qd * (QBW // P), (qd + 1) * (QBW // P)))
        for qc in pend_oproj:
            oproj(qc)

    nc.compile()
    return nc


_NC_CACHE = {}


def _get_nc(S, SKV, bf16=True):
    key = (S, SKV, bf16)
    if key not in _NC_CACHE:
        _NC_CACHE[key] = build_nc(S, SKV, bf16)
    return _NC_CACHE[key]


def _install_ntff_hook():
    try:
        mod = types.ModuleType("antenv.axon_hooks")
        state = {"hook": None}
        mod.set_axon_ntff_profile_hook = lambda h: state.__setitem__("hook", h)
        mod.get_axon_ntff_profile_hook = lambda: state["hook"]
        sys.modules["antenv.axon_hooks"] = mod
        from trn_agent_boot.trn_boot import _ntff_profile_via_ctypes

        mod.set_axon_ntff_profile_hook(
            _ntff_profile_via_ctypes("/opt/axon/libaxon_pjrt.so")
        )
        bass_utils.upload_artifacts = lambda tmpdir: "local://" + tmpdir
        return state["hook"] is not None
    except Exception:
        return False


def run_cores(in_maps, S=2048, SKV=1152, bf16=True, profile=False):
    nc = _get_nc(S, SKV, bf16)
    trace = bool(profile) and _install_ntff_hook()
    res = bass_utils.run_bass_kernel_spmd(
        nc, in_maps, core_ids=list(range(len(in_maps))), trace=trace
    )
    return res


def make_in_maps(q, k, v, mask, Wq, bq, Wk, bk, Wv, Wo, bf16=True):
    B, S, _ = q.shape
    mmd = ml_dtypes.bfloat16 if bf16 else np.float32
    q = np.asarray(q, np.float32)
    k = np.asarray(k, np.float32)
    v = np.asarray(v, np.float32)
    keep = ~np.asarray(mask).reshape(B, S)
    counts = keep.sum(axis=1)
    SKV = max(1024, int(-(-int(counts.max()) // 128)) * 128)
    Wq, Wk, Wv, Wo = (np.asarray(a, np.float32) for a in (Wq, Wk, Wv, Wo))
    bq, bk = np.asarray(bq, np.float32), np.asarray(bk, np.float32)
    in_maps = []
    NKT = SKV // 128
    for b in range(B):
        idx = np.nonzero(keep[b])[0]
        n = len(idx)
        kTc = np.zeros((D, SKV), np.float32)
        kTc[:, :n] = k[b][idx].T
        vTc = np.zeros((D, SKV), np.float32)
        vTc[:, :n] = v[b][idx].T
        mvec = np.zeros(SKV, np.float32)
        mvec[:n] = 1.0
        qTb = np.ascontiguousarray(q[b].T).astype(mmd)
        kTc = kTc.astype(mmd)
        vTc = vTc.astype(mmd)
        for half in range(2):
            hs = slice(DH * half, DH * (half + 1))
            # smalls: col 0..2 bq dt-tiles, 3..5 bk, 6.. mv k-tiles
            sm = np.zeros((128, 6 + NKT), np.float32)
            sm[:, 0:3] = bq[hs].reshape(3, 128).T
            sm[:, 3:6] = bk[hs].reshape(3, 128).T
            sm[:, 6:] = mvec.reshape(NKT, 128).T
            in_maps.append(
                {
                    "qT": qTb,
                    "kT": kTc,
                    "vT": vTc,
                    "wq": np.ascontiguousarray(Wq[:, hs]).astype(mmd),
                    "wk": np.ascontiguousarray(Wk[:, hs]).astype(mmd),
                    "wv": np.ascontiguousarray(Wv[:, hs]).astype(mmd),
                    "wo": np.ascontiguousarray(Wo[hs, :]).astype(mmd),
                    "smalls": sm,
                }
            )
    return in_maps, SKV


def kernel(q, k, v, mask, Wq, bq, Wk, bk, Wv, bv, Wo, bo):
    q = np.asarray(q, np.float32)
    B, S, _ = q.shape
    bf16 = os.environ.get("BASS_PRECISE") != "1"
    in_maps, SKV = make_in_maps(q, k, v, mask, Wq, bq, Wk, bk, Wv, Wo, bf16=bf16)
    res = run_cores(
        in_maps, S=S, SKV=SKV, bf16=bf16,
        profile=os.environ.get("BASS_PROFILE") == "1",
    )
    if os.environ.get("BASS_PROFILE") == "1" and res.exec_time_ns is not None:
        print(f"HW exec time: {res.exec_time_ns} ns")
    cvec = (
        np.asarray(bv, np.float32) @ np.asarray(Wo, np.float32)
        + np.asarray(bo, np.float32)
    )
    out = np.empty((B, S, D), np.float32)
    for b in range(B):
        out[b] = (
            np.asarray(res.results[2 * b]["out"], np.float32)
            + np.asarray(res.results[2 * b + 1]["out"], np.float32)
            + cvec
        )
    return out
